# revision 75
# baseline (speedup 1.0000x reference)
"""Low-rank multi-head attention Bass kernel for Trainium2 (8 NeuronCores).

Sharding: (batch, query-block) data parallel. 8 cores = 2 batches x 4 query
blocks. Each core receives TWO blocks of x host-transposed to feature-major
bf16 (own block + next neighbor), computes k1/v1 for both locally, and an
AllGather within each 4-core batch group carries only each core's FIRST
block (fp8 k1 + bf16 v1 bytes, u8 payload) -- so the collective launches
after a single kv pass and half the keys never touch the network. Keys are
consumed in per-core rolled order (local blocks first; softmax is invariant
to key order) so local attention fully covers the collective latency.

Performance structure:
  * Score matmuls run in fp8e4 with MatmulPerfMode.DoubleRow (0.5 PE
    cycles/row). The contraction is padded to 2x32 k-tiles, chunk-major so
    the two k-tiles of each key chunk are contiguous (dual-fp8 ldweights
    forbids gapped k-tile strides): kt0 = ranks 0..31 of k1, kt1 = [ones
    row, zeros...]. k1 rows are replicated into 4 partition bands so head h
    reads its lhsT at the same partition base as its band-stacked qh. qh is
    scaled by 16 (folded into wm) to center fp8 quantization; the exp scale
    absorbs the 1/16.
  * qh for FOUR heads is stacked in partition bands of one PSUM tile pair,
    evacuated with two 128-partition fp8 copies (4x fewer evac ops).
  * GPSIMD cannot touch PSUM, so exp (ACT exact / DVE Schraudolph i16
    bit-trick) and all PSUM evacuations are spread over ACT+DVE by a greedy
    least-loaded balancer; Pool handles SBUF-only work (memsets, epilogue
    sums, SWDGE DMAs).
  * ONE unified 4-slot rotating PSUM pool (2KB slots) plus 4 persistent
    accumulator banks, open for the whole kernel: no pool-close drain
    barriers, so qh prep, z-builds, local attention, the collective and
    tail prep all overlap freely. Scores/exp/attn@Z use a full 384-query
    stripe so every accumulator partition is written (finite pads).
  * attn@Z accumulates into 60 persistent PSUM slots ([query,34] per
    (head, query-chunk)); softmax division + the strided head-sum reduce
    fire per accumulator tile as soon as its last head completes.
  * Output is staged bf16 (host upcasts to f32), projected per-512-column
    chunk with small evacs/DMAs that pipeline through the tail.
"""

import os
import sys

sys.path.insert(0, "/opt/trn_rl_repo")

from contextlib import ExitStack

import numpy as np

import concourse.bass as bass
import concourse.tile as tile
from concourse import bacc
from concourse import mybir
from concourse.masks import make_identity

F32 = mybir.dt.float32
BF16 = mybir.dt.bfloat16
FP8 = mybir.dt.float8e4
U8 = mybir.dt.uint8
I16 = mybir.dt.int16
AF = mybir.ActivationFunctionType
ALU = mybir.AluOpType
DR = mybir.MatmulPerfMode.DoubleRow

H, D, R, N = 20, 64, 32, 1280

NCORES = 8
QP = 4  # query blocks per batch
SCALE = float(D) ** -0.5  # 0.125
QSC = 16.0  # qh pre-scale folded into wm (fp8 range centering)

# Schraudolph exp -> bf16 bits: i16 = round(x * s * 128/ln2 + (127*128 - C))
SCH_A = (SCALE / QSC) * 128.0 / float(np.log(2.0))
SCH_B = 127.0 * 128.0 - 0.0579848 * 128.0


def _chunks(total, size, start=0):
    out = []
    s = start
    while s < total:
        out.append((s, min(size, total - s)))
        s += size
    return out


def build_nc(S, SQ):
    nc = bacc.Bacc("TRN2", target_bir_lowering=False, debug=False, num_devices=NCORES)

    SQPAD = 128 * ((SQ + 127) // 128)
    xb = nc.dram_tensor("xb", [2 * SQPAD, N], BF16, kind="ExternalInput")
    Wq1 = nc.dram_tensor("Wq1", [R, N], F32, kind="ExternalInput")
    Wq2 = nc.dram_tensor("Wq2", [N, R], F32, kind="ExternalInput")
    bq = nc.dram_tensor("bq", [N], F32, kind="ExternalInput")
    Wk1 = nc.dram_tensor("Wk1", [R, N], F32, kind="ExternalInput")
    Wk2 = nc.dram_tensor("Wk2", [N, R], F32, kind="ExternalInput")
    bk = nc.dram_tensor("bk", [N], F32, kind="ExternalInput")
    Wv1 = nc.dram_tensor("Wv1", [R, N], F32, kind="ExternalInput")
    Wv2 = nc.dram_tensor("Wv2", [N, R], F32, kind="ExternalInput")
    bv = nc.dram_tensor("bv", [N], F32, kind="ExternalInput")
    Wo1 = nc.dram_tensor("Wo1", [R, N], F32, kind="ExternalInput")
    Wo2 = nc.dram_tensor("Wo2", [N, R], F32, kind="ExternalInput")
    bo = nc.dram_tensor("bo", [N], F32, kind="ExternalInput")
    out = nc.dram_tensor("out", [SQ, N], BF16, kind="ExternalOutput")

    SQP = SQ + (SQ % 2)                      # 376: even moving dim
    SQE = 128 * ((SQ + 127) // 128)          # 384: full az coverage width
    ICH = _chunks(SQ, 128)                   # query chunks (3)
    # key chunks: 3 per 375-row block (blocks never straddled); local first
    SCH = [(SQ * m + o, w) for m in range(QP) for (o, w) in _chunks(SQ, 128)]
    NJ = len(SCH)
    NIC = len(ICH)
    OSUB = _chunks(N, 512)
    NC10 = N // 128

    def mm(out_, lhsT, rhs, **kw):
        nc.tensor.matmul(out_, lhsT, rhs, **kw)

    # ---- greedy elementwise engine balancer (cost-model rates, ns).
    # GPSIMD/Pool cannot access PSUM, so PSUM-touching ops go to ACT/DVE.
    ew = {"act": 0.0, "dve": 0.0, "pool": 0.0}

    def ew_cost(eng, free, kind="copy"):
        if eng == "act":
            return free * 0.8333 + 185.0
        if eng == "dve":
            return free * 1.0417 + 125.0
        eff = 0.42 if kind == "add" else 0.6
        return free * 0.8333 / eff + 95.0

    def ew_pick(free, allowed=("act", "dve"), kind="copy"):
        e = min(allowed, key=lambda k: ew[k] + ew_cost(k, free, kind))
        ew[e] += ew_cost(e, free, kind)
        return e

    def bal_copy(dst, src, free, allowed=("act", "dve")):
        e = ew_pick(free, allowed)
        if e == "act":
            nc.scalar.copy(dst, src)
        elif e == "dve":
            nc.vector.tensor_copy(dst, src)
        else:
            nc.gpsimd.tensor_copy(dst, src)

    def bal_scale(dst, src, mult, free):
        e = ew_pick(free)
        if e == "act":
            nc.scalar.activation(dst, src, AF.Copy, scale=mult)
        else:
            nc.vector.tensor_scalar(dst, src, mult, None, ALU.mult)

    def bal_exp(atv, scv, free):
        e = ew_pick(free)
        if e == "act":
            nc.scalar.activation(atv, scv, AF.Exp, scale=SCALE / QSC)
        else:
            nc.vector.tensor_scalar(
                atv.bitcast(I16), scv, SCH_A, SCH_B, ALU.mult, ALU.add
            )

    def bal_tt(dst, a, b, free, op=ALU.mult, allowed=("dve", "pool")):
        e = ew_pick(free, allowed, kind="add" if op == ALU.add else "copy")
        if e == "dve":
            nc.vector.tensor_tensor(dst, a, b, op)
        else:
            nc.gpsimd.tensor_tensor(dst, a, b, op)

    with tile.TileContext(nc) as tc, ExitStack() as ctx:
        wp = ctx.enter_context(tc.tile_pool(name="wp", bufs=1))
        small_p = ctx.enter_context(tc.tile_pool(name="small_p", bufs=8))
        at2_p = ctx.enter_context(tc.tile_pool(name="at2_p", bufs=24))
        outp = ctx.enter_context(tc.tile_pool(name="outp", bufs=4))
        # ---- unified PSUM: 4 persistent accumulator banks + ONE 4-slot
        # rotating pool (2KB slots) shared by every transient PSUM tile.
        # No setup/attention phase split -> no drain barrier: local
        # attention overlaps the collective and the qh prep.
        psC = ExitStack()
        ps_acc = psC.enter_context(tc.tile_pool(name="ps_acc", bufs=1, space="PSUM"))
        ps_all = psC.enter_context(tc.tile_pool(name="ps_all", bufs=4, space="PSUM"))
        dramp = ctx.enter_context(tc.tile_pool(name="dramp", bufs=1, space="DRAM"))

        accT = [
            ps_acc.tile([128, 510], F32, name=f"acc{t}", tag=f"acc{t}")
            for t in range(4)
        ]

        def pst(shape):
            return ps_all.tile(shape, F32, tag="sc", name="pst")

        # ---- persistent SBUF tensors ----
        ident = wp.tile([128, 128], F32)
        make_identity(nc, ident[:])

        WkvT = wp.tile([128, 640], BF16)     # [Wk1.T | Wv1.T] interleaved per chunk
        WqT = wp.tile([128, 320], BF16)
        WoT = wp.tile([128, 320], BF16)      # Wo1.T chunks
        Wv2c = wp.tile([64, 640], BF16)      # Wv2 head-major: [d, 32h+r]
        WoT2 = wp.tile([64, 640], BF16)      # Wo1.T head-major: [d, 32h+s]
        Wq2aug = wp.tile([64, 660], F32)     # [Wq2 | bq] head-major, d on partitions
        Wk2aug = wp.tile([64, 660], F32)
        bk2 = wp.tile([64, 640], F32)        # per head: col 0 = bk_h, cols 1:32 zero
        bv_c = wp.tile([128, 12], BF16)
        bo_row = wp.tile([1, N], F32)
        Wo2Ta = wp.tile([33, N], BF16)       # rows 0:32 Wo2.T, row 32 = Wo2@Wo1@bv + bo
        t1sb = wp.tile([32, 2], BF16)
        wsb2 = wp.tile([128, 320], F32)

        # k1 in DoubleRow fp8 layout, chunk-major so the two k-tiles of each
        # key chunk are CONTIGUOUS (ISA: dual-fp8 ldweights forbids gapped
        # k-tile strides): chunk c at cols 256c = [kt0(128 keys) | kt1(128)].
        # kt0 = ranks 0..31 of k1; kt1 = [ones row, zeros...]; chunk 2 key
        # cols 119:128 are zero pads (their scores are finite junk, never
        # consumed by attn@Z which slices [:p]). The 32 rows are REPLICATED
        # into 4 partition bands (32g) so head h can read its lhsT at the
        # same partition base as its qh band (quad-stacked qh evac).
        k1f8 = [
            wp.tile([128, 256 * NIC], FP8, name=f"k1f8{m}", tag=f"k1f8{m}")
            for m in range(QP)
        ]
        k1sb = [
            wp.tile([32, SQP], FP8, name=f"k1sb{bi}", tag=f"k1sb{bi}")
            for bi in range(2)
        ]  # rank-major k1 staging for the cc (two local blocks)
        v1b = [
            wp.tile([32, SQP], BF16, name=f"v1b{m}", tag=f"v1b{m}") for m in range(QP)
        ]
        Mall = wp.tile([32, 640], BF16)      # Mh side by side
        q1Ta = wp.tile([33, SQP], BF16)      # rows 0:32 q1T, row 32 ones; col SQ zero
        o1Ta = wp.tile([33, SQP], BF16)
        Zt = [wp.tile([128, 680], BF16, name=f"Zt{j}", tag=f"Zt{j}") for j in range(NJ)]
        # Zt layout: 20 heads x [32 z-cols | 1.0 | 0.0]; the ones col feeds
        # the softmax denominator through the same accumulating matmul.
        qh_f8 = [
            wp.tile([128, 2 * SQE], FP8, name=f"qh{hp}", tag=f"qh{hp}")
            for hp in range(H // 4)
        ]
        Yg = [wp.tile([128, 480], F32, name=f"Yg{t}", tag=f"Yg{t}") for t in range(4)]
        rrt = wp.tile([128, 64], F32)        # reciprocals, 16 per group
        o1grp = wp.tile([128, 96], F32)      # [i, ic, r] final o1 (pre-transpose)
        scr = wp.tile([128, 384], F32)       # reduce scratch (4 x 96)


        def kw(m, c, g):
            # DoubleRow lhsT for block m, key chunk c, band g: [32, 2, 128]
            return k1f8[m][32 * g : 32 * g + 32, 256 * c : 256 * c + 256].rearrange(
                "p (two s) -> p two s", s=128
            )

        def q3(h):
            hp, g = h // 4, h % 4
            return qh_f8[hp][32 * g : 32 * g + 32, :].rearrange(
                "p (two s) -> p two s", s=SQE
            )

        def k_fill_band0(m, eng, eng2):
            # chunk-major band 0 from the rank-major staging tile (SBUF DMA)
            eng.dma_start(
                k1f8[m][0:32, 0:512].rearrange("p (c s) -> p c s", s=256)[
                    :, :, 0:128
                ],
                k1sb[m][:, 0:256].rearrange("p (c s) -> p c s", s=128),
            )
            eng2.dma_start(k1f8[m][0:32, 512 : 512 + 119], k1sb[m][:, 256:SQ])

        def k_replicate(m, eng):
            # copy band 0 (incl kt1 ones/zeros) into bands 1..3
            for g in range(1, QP):
                eng.dma_start(
                    k1f8[m][32 * g : 32 * g + 32, :], k1f8[m][0:32, :]
                )

        def build_z(j):
            j0, p = SCH[j]
            mp_, off_ = j0 // SQ, j0 % SQ
            for half in range(2):
                zp = pst([128, 512])
                mm(
                    zp[:p, 0:320],
                    v1b[mp_][:, off_ : off_ + p],
                    Mall[:, 320 * half : 320 * half + 320],
                )
                zdst = Zt[j][:p, 340 * half : 340 * half + 340].rearrange(
                    "p (h c) -> p h c", c=34
                )[:, :, 0:32]
                zsrc = zp[:p, 0:320].rearrange("p (h c) -> p h c", c=32)
                bal_copy(zdst, zsrc, 320)

        # ================= setup: x, k/v, collective =================
        with tc.tile_pool(name="wload", bufs=3) as wload, tc.tile_pool(
            name="xT_p", bufs=1
        ) as xT_p:
            # x^T: one tile PER query chunk (separate tiles -> the three XBAR
            # transposes are independent, no WAW serialization). xTa[ic] =
            # [128 f-partitions, 10 c-blocks x 128 queries]; x arrives bf16
            # (host cast, padded to 384 rows) so the XBAR reads DRAM.
            xTa = [
                xT_p.tile([128, N], BF16, name=f"xTa{ic}", tag=f"xTa{ic}")
                for ic in range(2 * NIC)
            ]
            with tc.high_priority():
                # x arrives HOST-TRANSPOSED (feature-major, bf16): row
                # 128*ic+p holds [c, s] -> x[128*ic+s, 128*c+p], for TWO
                # query blocks (own + next neighbor). Plain DMAs, no XBAR.
                # The k/v weight loads go FIRST on the serial DMA device so
                # the WkvT transposes (PE) run DURING the x transfers; then
                # block A chunks; block B + q/o weights follow later.
                wsb_k = wload.tile([32, N], F32, tag="wsb_k")
                nc.sync.dma_start(wsb_k[:], Wk1[:])
                wsb_v = wload.tile([32, N], F32, tag="wsb_v")
                nc.scalar.dma_start(wsb_v[:], Wv1[:])
                for ic in range(NIC):
                    eng = (nc.sync, nc.scalar)[ic % 2]
                    eng.dma_start(xTa[ic][:, :], xb[128 * ic : 128 * ic + 128, :])
            for ic in range(NIC, 2 * NIC):
                eng = (nc.sync, nc.scalar)[ic % 2]
                eng.dma_start(xTa[ic][:, :], xb[128 * ic : 128 * ic + 128, :])

            wsb_q = wload.tile([32, N], F32, tag="wsb_q")
            wsb_o = wload.tile([32, N], F32, tag="wsb_o")

            def xmov(bi, ic, c):
                # moving operand: queries 128*ic.. of feature chunk c
                w = min(128, SQP - 128 * ic)
                return xTa[NIC * bi + ic][:, 128 * c : 128 * c + w]

            # PE p-state warm-up
            warm_ps = pst([128, 320])
            for _ in range(18):
                mm(warm_ps[:, 0:128], ident[:], ident[:], is_transpose=True)

            # --- Wk1/Wv1 -> WkvT (f32 transposes, bf16 evacs) ---
            with tc.high_priority():
                for wi, wsb in enumerate((wsb_k, wsb_v)):
                    tp = pst([128, 320])
                    for c in range(NC10):
                        mm(
                            tp[:, 32 * c : 32 * c + 32],
                            wsb[:, 128 * c : 128 * c + 128],
                            ident[:32, :32],
                            is_transpose=True,
                        )
                    dst = WkvT[:, :].rearrange(
                        "p (c two r) -> p c two r", two=2, r=32
                    )[:, :, wi]
                    bal_copy(dst, tp[:, :].rearrange("p (c r) -> p c r", r=32), 320)

                # --- local k1/v1, block A first ([64, SQP] psum: k rows
                # 0:32, v 32:64). Moving split per (c, ic): each matmul
                # waits only ONE xTa. Only block A feeds the collective --
                # block B (qi+1) is computed locally by every core, so the
                # gather carries just one block per core and can launch
                # after a single kv pass.
                cc_in = dramp.tile([96, SQ], U8)
                cc_out = dramp.tile([96 * QP, SQ], U8)
                for bi in range(2):
                    kv_ps = pst([64, SQP])
                    for c in range(NC10):
                        for ic in range(NIC):
                            w = min(128, SQP - 128 * ic)
                            mm(
                                kv_ps[:, 128 * ic : 128 * ic + w],
                                WkvT[:, 64 * c : 64 * c + 64],
                                xmov(bi, ic, c),
                                start=(c == 0 and ic == 0),
                                stop=(c == NC10 - 1 and ic == NIC - 1),
                                skip_group_check=True,
                            )
                    # staging evacs, single-writer tiles -> clean cc deps
                    nc.vector.tensor_copy(k1sb[bi][:, 0:SQ], kv_ps[0:32, 0:SQ])
                    nc.scalar.copy(v1b[bi][:, 0:SQ], kv_ps[32:64, 0:SQ])
                    if bi == 0:
                        nc.sync.dma_start(
                            cc_in[0:32, :], k1sb[0][:, 0:SQ].bitcast(U8)
                        )
                        nc.scalar.dma_start(
                            cc_in[32:96, :], v1b[0][:, 0:SQ].bitcast(U8)
                        )
                        nc.gpsimd.collective_compute(
                            "AllGather",
                            ALU.bypass,
                            replica_groups=[
                                list(range(g * QP, (g + 1) * QP))
                                for g in range(NCORES // QP)
                            ],
                            ins=[cc_in[:].opt()],
                            outs=[cc_out[:].opt()],
                        )

            nc.sync.dma_start(wsb_q[:], Wq1[:])
            nc.scalar.dma_start(wsb_o[:], Wo1[:])
            # local k1f8 band 0 + kt1 memsets + replicate
            k_fill_band0(0, nc.sync, nc.sync)
            k_fill_band0(1, nc.scalar, nc.scalar)
            for m in range(QP):
                kt1v = k1f8[m][0:32, :].rearrange("p (c s) -> p c s", s=256)[
                    :, :, 128:256
                ]
                nc.gpsimd.memset(kt1v, 0.0)
                nc.gpsimd.memset(kt1v[0:1], 1.0)
                nc.gpsimd.memset(k1f8[m][0:32, 512 + 119 : 512 + 128], 0.0)
            k_replicate(0, nc.sync)
            k_replicate(1, nc.scalar)

            # --- gpsimd work queued behind the collective issue ---
            nc.gpsimd.dma_start(
                Wv2c[:, :].rearrange("d (h r) -> d h r", r=32),
                Wv2[:].rearrange("(h d) r -> d h r", d=64),
            )
            nc.gpsimd.memset(bk2[:], 0.0)
            nc.gpsimd.memset(q1Ta[32:33, :], 1.0)
            if SQP > SQ:
                nc.gpsimd.memset(q1Ta[0:33, SQ:SQP], 0.0)
            for j in range(NJ):
                z3 = Zt[j][:, :].rearrange("p (h c) -> p h c", c=34)
                nc.gpsimd.memset(z3[:, :, 32:33], 1.0)
                nc.gpsimd.memset(z3[:, :, 33:34], 0.0)
            for hp in range(H // 4):
                q4 = qh_f8[hp][:, :].rearrange("p (two s) -> p two s", s=SQE)
                nc.gpsimd.memset(q4[:, :, SQP:SQE], 0.0)
            nc.gpsimd.memset(o1Ta[32:33, :], 1.0)
            if SQP > SQ:
                nc.gpsimd.memset(o1Ta[0:33, SQ:SQP], 0.0)
            nc.gpsimd.memset(bv_c[:], 0.0)
            nc.gpsimd.dma_start(
                bv_c[:, 0:10], bv[:].rearrange("(c p) -> p c", p=128)
            )

            nc.sync.dma_start(
                Wq2aug[:, :].rearrange("d (h r) -> d h r", r=33)[:, :, 0:32],
                Wq2[:].rearrange("(h d) r -> d h r", d=64),
            )
            nc.scalar.dma_start(
                Wk2aug[:, :].rearrange("d (h r) -> d h r", r=33)[:, :, 0:32],
                Wk2[:].rearrange("(h d) r -> d h r", d=64),
            )
            nc.sync.dma_start(
                Wq2aug[:, :].rearrange("d (h r) -> d h r", r=33)[:, :, 32:33],
                bq[:].rearrange("(h d) -> d h", d=64).unsqueeze(2),
            )
            nc.scalar.dma_start(
                Wk2aug[:, :].rearrange("d (h r) -> d h r", r=33)[:, :, 32:33],
                bk[:].rearrange("(h d) -> d h", d=64).unsqueeze(2),
            )
            bal_copy(
                bk2[:, :].rearrange("d (h c) -> d h c", c=32)[:, :, 0:1],
                Wk2aug[:, :].rearrange("d (h r) -> d h r", r=33)[:, :, 32:33],
                20,
            )
            # remote k1/v1: gathered block (qi+mp)%4 at cc rows b*96
            qi = nc.sync.partition_id() % QP
            qi2 = nc.scalar.partition_id() % QP
            for mp in range(2, QP):
                b = (qi + mp) % QP
                b2 = (qi2 + mp) % QP
                nc.sync.dma_start(
                    k1f8[mp][0:32, 0:512].rearrange("p (c s) -> p c s", s=256)[
                        :, :, 0:128
                    ].bitcast(U8),
                    cc_out[bass.DynSlice(b * 96, 32), 0:256],
                )
                nc.sync.dma_start(
                    k1f8[mp][0:32, 512 : 512 + 119].bitcast(U8),
                    cc_out[bass.DynSlice(b * 96, 32), 256:SQ],
                )
                k_replicate(mp, nc.sync)
                nc.scalar.dma_start(
                    v1b[mp][:, 0:SQ].bitcast(U8),
                    cc_out[bass.DynSlice(b2 * 96 + 32, 64), :],
                )

            nc.sync.dma_start(bo_row[:], bo[:].unsqueeze(0))
            nc.sync.dma_start(
                wsb2[:].rearrange("p (c r) -> p c r", r=32),
                Wo2[:].rearrange("(c p) r -> p c r", p=128),
            )

            # --- q-side prep ---
            tp = pst([128, 320])
            for c in range(NC10):
                mm(
                    tp[:, 32 * c : 32 * c + 32],
                    wsb_q[:, 128 * c : 128 * c + 128],
                    ident[:32, :32],
                    is_transpose=True,
                )
            bal_copy(WqT[:], tp[:], 320)
            q1ps = pst([32, SQP])
            for c in range(NC10):
                for ic in range(NIC):
                    w = min(128, SQP - 128 * ic)
                    mm(
                        q1ps[:, 128 * ic : 128 * ic + w],
                        WqT[:, 32 * c : 32 * c + 32],
                        xmov(0, ic, c),
                        start=(c == 0 and ic == 0),
                        stop=(c == NC10 - 1 and ic == NIC - 1),
                        skip_group_check=True,
                    )
            nc.vector.tensor_copy(q1Ta[0:32, :], q1ps[:])

            # --- Mall: Mh = Wv2h @ Wo1_h.T ---
            tp = pst([128, 320])
            for c in range(NC10):
                mm(
                    tp[:, 32 * c : 32 * c + 32],
                    wsb_o[:, 128 * c : 128 * c + 128],
                    ident[:32, :32],
                    is_transpose=True,
                )
            bal_copy(WoT[:], tp[:], 320)
            # head-major copy for the Mall matmuls (all partitions 0:64)
            w2v = WoT2[:, :].rearrange("d (hc two r) -> d hc two r", two=2, r=32)
            t3v = tp[:, :].rearrange("p (c r) -> p c r", r=32)
            nc.vector.tensor_copy(w2v[:, :, 0], t3v[0:64, :, :])
            nc.scalar.copy(w2v[:, :, 1], t3v[64:128, :, :])
            for half in range(2):
                mall_ps = pst([32, 320])
                for hh in range(10):
                    h = 10 * half + hh
                    mm(
                        mall_ps[:, 32 * hh : 32 * hh + 32],
                        Wv2c[:, 32 * h : 32 * h + 32],
                        WoT2[:, 32 * h : 32 * h + 32],
                        skip_group_check=True,
                    )
                bal_copy(Mall[:, 320 * half : 320 * half + 320], mall_ps[:], 320)

            # local Z chunks: both local blocks (need Mall + local v1 only)
            for j in range(2 * NIC):
                build_z(j)

            def attend(J, h):
                j0, p = SCH[J]
                mp = j0 // SQ
                # full 128-key lhsT even for the 119-wide chunk: pad keys
                # are zero -> finite junk scores in rows p:128, unused
                sc = pst([128, 512])
                mm(
                    sc[0:128, 0:SQE],
                    kw(mp, J % NIC, h % 4),
                    q3(h)[:, :, :],
                    perf_mode=DR,
                    tile_position=(32 * (h % 4), 0),
                )
                at2 = at2_p.tile([128, SQE], BF16, tag="at")
                bal_exp(at2[:p, :], sc[:p, 0:SQE], SQE)
                t, base = h // 5, (h % 5) * 3
                for ic, (i0, iw) in enumerate(ICH):
                    c0 = 34 * (base + ic)
                    # az reads a full 128-query stripe (query pads are
                    # finite junk) so every acc partition gets written
                    mm(
                        accT[t][0:128, c0 : c0 + 34],
                        at2[:p, i0 : i0 + 128],
                        Zt[J][:p, 34 * h : 34 * h + 34],
                        start=(J == 0 and h % 5 == 0 and ic == 0),
                        stop=(J == NJ - 1 and h % 5 == 4 and ic == 2),
                        skip_group_check=True,
                    )
                if J == NJ - 1 and h % 5 == 4:
                    # tile h//5 complete: divide by the softmax denominator
                    # now, while the remaining exps still run
                    td = h // 5
                    acc3 = accT[td][:, :].rearrange("p (s c) -> p s c", c=34)
                    nc.vector.reciprocal(
                        rrt[:, 16 * td : 16 * td + 15].unsqueeze(2),
                        acc3[:, :, 32:33],
                    )
                    ew["dve"] += 141.0
                    y_ap = Yg[td][:, :].rearrange("p (s c) -> p s c", c=32)
                    rb, ab = bass.broadcast_tensor_aps(
                        rrt[:, 16 * td : 16 * td + 15].unsqueeze(2),
                        acc3[:, :, 0:32],
                    )
                    bal_tt(y_ap, ab, rb, 480, ALU.mult, allowed=("dve",))
                    # head-sum partial for this group tile, immediately
                    rsrc = Yg[td][:, :].rearrange(
                        "p (hl ic c) -> p ic c hl", hl=5, ic=3, c=32
                    )
                    rdst = scr[:, 96 * td : 96 * td + 96].rearrange(
                        "p (ic c) -> p ic c", c=32
                    )
                    nc.vector.tensor_reduce(
                        rdst, rsrc, mybir.AxisListType.X, ALU.add
                    )
                    ew["dve"] += 625.0

            # --- wm + qh (four heads stacked in partition bands 32g), each
            # quad IMMEDIATELY followed by its local-block attention so the
            # exp engines fill the collective window ---
            for hp in range(H // 4):
                # phase 1: the four wm coefficient tiles (short-lived psum)
                wms = []
                for g in range(4):
                    h = 4 * hp + g
                    wmps = pst([33, 64])
                    mm(
                        wmps[0:32, 0:32],
                        Wq2aug[:, 33 * h : 33 * h + 32],
                        Wk2aug[:, 33 * h : 33 * h + 32],
                    )
                    mm(
                        wmps[32:33, 0:32],
                        Wq2aug[:, 33 * h + 32 : 33 * h + 33],
                        Wk2aug[:, 33 * h : 33 * h + 32],
                        skip_group_check=True,
                    )
                    # cols 32:64 = kt1 coefficients: col 32 = bk (ones-row
                    # coefficient), cols 33:64 = 0 (zero cols of bk2)
                    mm(
                        wmps[0:33, 32:64],
                        Wq2aug[:, 33 * h : 33 * h + 33],
                        bk2[:, 32 * h : 32 * h + 32],
                        skip_group_check=True,
                    )
                    wm = small_p.tile([33, 64], BF16, tag="wm_sb")
                    bal_scale(wm[:], wmps[:], QSC, 64)
                    wms.append(wm)
                # phase 2: band-stacked qh matmuls + two 128-partition evacs
                qhA = pst([128, 512])
                qhB = pst([128, 512])
                for g in range(4):
                    mm(
                        qhA[32 * g : 32 * g + 32, 0:SQP],
                        wms[g][:, 0:32],
                        q1Ta[:],
                        skip_group_check=(g > 0),
                        tile_position=(0, 32 * g),
                    )
                    mm(
                        qhB[32 * g : 32 * g + 32, 0:SQP],
                        wms[g][:, 32:64],
                        q1Ta[:],
                        skip_group_check=(g > 0),
                        tile_position=(0, 32 * g),
                    )
                qdst = qh_f8[hp][:, :].rearrange("p (two s) -> p two s", s=SQE)
                bal_copy(qdst[:, 0, 0:SQP], qhA[:, 0:SQP], SQP)
                bal_copy(qdst[:, 1, 0:SQP], qhB[:, 0:SQP], SQP)

            # local attention en bloc (quads all ready; scores get the full
            # 4-slot rotation depth)
            for hp in range(H // 4):
                for J in range(2 * NIC):
                    for g in range(4):
                        attend(J, 4 * hp + g)

            # --- Wo2 -> Wo2Ta (bf16) + bo_eff into row 32 (tail prep) ---
            for g0 in range(0, NC10, 4):
                gn = min(4, NC10 - g0)
                tp2 = pst([32, 512])
                for k in range(gn):
                    c = g0 + k
                    mm(
                        tp2[:, 128 * k : 128 * k + 128],
                        wsb2[:, 32 * c : 32 * c + 32],
                        ident[:],
                        is_transpose=True,
                    )
                bal_copy(
                    Wo2Ta[0:32, 128 * g0 : 128 * (g0 + gn)],
                    tp2[:, : 128 * gn],
                    512,
                )
            t1ps = pst([32, 2])
            for c in range(NC10):
                mm(
                    t1ps[:],
                    WoT[:, 32 * c : 32 * c + 32],
                    bv_c[:, c : c + 2],
                    start=(c == 0),
                    stop=(c == NC10 - 1),
                )
            nc.vector.tensor_copy(t1sb[:], t1ps[:])
            for (n0, nw) in OSUB:
                beps = pst([1, 512])
                mm(beps[:, :nw], t1sb[:, 0:1], Wo2Ta[0:32, n0 : n0 + nw])
                nc.vector.tensor_add(
                    Wo2Ta[32:33, n0 : n0 + nw],
                    beps[:, :nw],
                    bo_row[:, n0 : n0 + nw],
                )

        # ================= remote attention =================
        for J in range(2 * NIC, NJ):
            if J == 2 * NIC:
                build_z(2 * NIC)
                build_z(2 * NIC + 1)
            if J + 2 < NJ:
                build_z(J + 2)
            for h in range(H):
                attend(J, h)

        # ================= tail: epilogue + out-projection ======
        psC.close()
        psD = ExitStack()
        psf = psD.enter_context(tc.tile_pool(name="psf", bufs=2, space="PSUM"))

        # add the 4 group partials -> o1grp [i, (ic, r)]
        bal_tt(scr[:, 0:96], scr[:, 0:96], scr[:, 96:192], 96, ALU.add)
        bal_tt(scr[:, 192:288], scr[:, 192:288], scr[:, 288:384], 96, ALU.add)
        bal_tt(o1grp[:], scr[:, 0:96], scr[:, 192:288], 96, ALU.add)

        # out-projection: per-OSUB pipeline (small evacs + small DMAs so
        # transfers overlap compute), bf16 staging halves the DMA bytes
        for ic, (i0, iw) in enumerate(ICH):
            o1ps = psf.tile([32, SQP], F32, tag="o1t", bufs=3)
            mm(
                o1ps[:, i0 : i0 + iw],
                o1grp[0:iw, 32 * ic : 32 * ic + 32],
                ident[:iw, :iw],
                is_transpose=True,
            )
            bal_copy(o1Ta[0:32, i0 : i0 + iw], o1ps[:, i0 : i0 + iw], iw)
            osb = outp.tile([128, N], BF16, tag="osb")
            for oi, (n0, nw) in enumerate(OSUB):
                fps = psf.tile([128, 512], F32, tag="fp", bufs=5)
                mm(fps[:iw, :nw], o1Ta[:, i0 : i0 + iw], Wo2Ta[:, n0 : n0 + nw])
                bal_copy(osb[:iw, n0 : n0 + nw], fps[:iw, :nw], nw)
                eng = (nc.sync, nc.gpsimd, nc.scalar)[(3 * ic + oi) % 3]
                eng.dma_start(out[i0 : i0 + iw, n0 : n0 + nw], osb[:iw, n0 : n0 + nw])

        psD.close()

    nc.compile()
    return nc


_NC_CACHE = {}


def _get_nc(S, SQ):
    key = (S, SQ)
    if key not in _NC_CACHE:
        _NC_CACHE[key] = build_nc(S, SQ)
    return _NC_CACHE[key]


def kernel(**inputs):
    from concourse.bass_utils import run_bass_kernel_spmd

    x = np.asarray(inputs["x"], dtype=np.float32)
    B, S, n = x.shape
    assert n == N and B * QP == NCORES
    SQ = S // QP
    nc = _get_nc(S, SQ)

    wnames = [
        "Wq1", "Wq2", "bq", "Wk1", "Wk2", "bk",
        "Wv1", "Wv2", "bv", "Wo1", "Wo2", "bo",
    ]
    weights = {
        k: np.ascontiguousarray(np.asarray(inputs[k], dtype=np.float32))
        for k in wnames
    }

    import ml_dtypes

    SQPAD = 128 * ((SQ + 127) // 128)
    NICW = SQPAD // 128
    in_maps = []
    for core in range(NCORES):
        b, qi = divmod(core, QP)
        xpad = np.zeros((2 * SQPAD, N), dtype=np.float32)
        xpad[:SQ] = x[b, SQ * qi : SQ * (qi + 1)]
        q2 = (qi + 1) % QP
        xpad[SQPAD : SQPAD + SQ] = x[b, SQ * q2 : SQ * (q2 + 1)]
        # feature-major host transpose: xbt[128*ic + p, 128*c + s]
        # = x[128*ic + s, 128*c + p]  (see xTa layout in build_nc)
        xbt = (
            xpad.reshape(2 * NICW, 128, N // 128, 128)
            .transpose(0, 3, 2, 1)
            .reshape(2 * SQPAD, N)
            .astype(ml_dtypes.bfloat16)
        )
        m = {"xb": np.ascontiguousarray(xbt)}
        m.update(weights)
        in_maps.append(m)

    res = run_bass_kernel_spmd(nc, in_maps, core_ids=list(range(NCORES)))
    outs = res.results if hasattr(res, "results") else res

    out = np.zeros((B, S, N), dtype=np.float32)
    for core in range(NCORES):
        b, qi = divmod(core, QP)
        out[b, SQ * qi : SQ * (qi + 1), :] = outs[core]["out"].astype(np.float32)
    return out


# revision 77
# speedup vs baseline: 1.0346x; 1.0346x over previous
"""Low-rank multi-head attention Bass kernel for Trainium2 (8 NeuronCores).

Sharding: (batch, query-block) data parallel. 8 cores = 2 batches x 4 query
blocks. Each core receives TWO blocks of x host-transposed to feature-major
bf16 (own block + next neighbor), computes k1/v1 for both locally, and an
AllGather within each 4-core batch group carries only each core's FIRST
block (fp8 k1 + bf16 v1 bytes, u8 payload) -- so the collective launches
after a single kv pass and half the keys never touch the network. Keys are
consumed in per-core rolled order (local blocks first; softmax is invariant
to key order) so local attention fully covers the collective latency.

Performance structure:
  * Score matmuls run in fp8e4 with MatmulPerfMode.DoubleRow (0.5 PE
    cycles/row). The contraction is padded to 2x32 k-tiles, chunk-major so
    the two k-tiles of each key chunk are contiguous (dual-fp8 ldweights
    forbids gapped k-tile strides): kt0 = ranks 0..31 of k1, kt1 = [ones
    row, zeros...]. k1 rows are replicated into 4 partition bands so head h
    reads its lhsT at the same partition base as its band-stacked qh. qh is
    scaled by 16 (folded into wm) to center fp8 quantization; the exp scale
    absorbs the 1/16.
  * qh for FOUR heads is stacked in partition bands of one PSUM tile pair,
    evacuated with two 128-partition fp8 copies (4x fewer evac ops).
  * GPSIMD cannot touch PSUM, so exp (ACT exact / DVE Schraudolph i16
    bit-trick) and all PSUM evacuations are spread over ACT+DVE by a greedy
    least-loaded balancer; Pool handles SBUF-only work (memsets, epilogue
    sums, SWDGE DMAs).
  * ONE unified 4-slot rotating PSUM pool (2KB slots) plus 4 persistent
    accumulator banks, open for the whole kernel: no pool-close drain
    barriers, so qh prep, z-builds, local attention, the collective and
    tail prep all overlap freely. Scores/exp/attn@Z use a full 384-query
    stripe so every accumulator partition is written (finite pads).
  * attn@Z accumulates into 60 persistent PSUM slots ([query,34] per
    (head, query-chunk)); softmax division + the strided head-sum reduce
    fire per accumulator tile as soon as its last head completes.
  * Output is staged bf16 (host upcasts to f32), projected per-512-column
    chunk with small evacs/DMAs that pipeline through the tail.
"""

import os
import sys

sys.path.insert(0, "/opt/trn_rl_repo")

from contextlib import ExitStack

import numpy as np

import concourse.bass as bass
import concourse.tile as tile
from concourse import bacc
from concourse import mybir
from concourse.masks import make_identity

F32 = mybir.dt.float32
BF16 = mybir.dt.bfloat16
FP8 = mybir.dt.float8e4
U8 = mybir.dt.uint8
I16 = mybir.dt.int16
AF = mybir.ActivationFunctionType
ALU = mybir.AluOpType
DR = mybir.MatmulPerfMode.DoubleRow

H, D, R, N = 20, 64, 32, 1280

NCORES = 8
QP = 4  # query blocks per batch
SCALE = float(D) ** -0.5  # 0.125
QSC = 16.0  # qh pre-scale folded into wm (fp8 range centering)

# Schraudolph exp -> bf16 bits: i16 = round(x * s * 128/ln2 + (127*128 - C))
SCH_A = (SCALE / QSC) * 128.0 / float(np.log(2.0))
SCH_B = 127.0 * 128.0 - 0.0579848 * 128.0


def _chunks(total, size, start=0):
    out = []
    s = start
    while s < total:
        out.append((s, min(size, total - s)))
        s += size
    return out


def build_nc(S, SQ):
    nc = bacc.Bacc("TRN2", target_bir_lowering=False, debug=False, num_devices=NCORES)

    SQPAD = 128 * ((SQ + 127) // 128)
    xb = nc.dram_tensor("xb", [2 * SQPAD, N], BF16, kind="ExternalInput")
    Wq1 = nc.dram_tensor("Wq1", [R, N], F32, kind="ExternalInput")
    Wq2 = nc.dram_tensor("Wq2", [N, R], F32, kind="ExternalInput")
    bq = nc.dram_tensor("bq", [N], F32, kind="ExternalInput")
    Wk1 = nc.dram_tensor("Wk1", [R, N], F32, kind="ExternalInput")
    Wk2 = nc.dram_tensor("Wk2", [N, R], F32, kind="ExternalInput")
    bk = nc.dram_tensor("bk", [N], F32, kind="ExternalInput")
    Wv1 = nc.dram_tensor("Wv1", [R, N], F32, kind="ExternalInput")
    Wv2 = nc.dram_tensor("Wv2", [N, R], F32, kind="ExternalInput")
    bv = nc.dram_tensor("bv", [N], F32, kind="ExternalInput")
    Wo1 = nc.dram_tensor("Wo1", [R, N], F32, kind="ExternalInput")
    Wo2 = nc.dram_tensor("Wo2", [N, R], F32, kind="ExternalInput")
    bo = nc.dram_tensor("bo", [N], F32, kind="ExternalInput")
    out = nc.dram_tensor("out", [SQ, N], BF16, kind="ExternalOutput")

    SQP = SQ + (SQ % 2)                      # 376: even moving dim
    SQE = 128 * ((SQ + 127) // 128)          # 384: full az coverage width
    ICH = _chunks(SQ, 128)                   # query chunks (3)
    # key chunks: 3 per 375-row block (blocks never straddled); local first
    SCH = [(SQ * m + o, w) for m in range(QP) for (o, w) in _chunks(SQ, 128)]
    NJ = len(SCH)
    NIC = len(ICH)
    OSUB = _chunks(N, 512)
    NC10 = N // 128

    def mm(out_, lhsT, rhs, **kw):
        nc.tensor.matmul(out_, lhsT, rhs, **kw)

    # ---- greedy elementwise engine balancer (cost-model rates, ns).
    # GPSIMD/Pool cannot access PSUM, so PSUM-touching ops go to ACT/DVE.
    ew = {"act": 0.0, "dve": 0.0, "pool": 0.0}

    def ew_cost(eng, free, kind="copy"):
        if eng == "act":
            return free * 0.8333 + 185.0
        if eng == "dve":
            return free * 1.0417 + 125.0
        eff = 0.42 if kind == "add" else 0.6
        return free * 0.8333 / eff + 95.0

    def ew_pick(free, allowed=("act", "dve"), kind="copy"):
        e = min(allowed, key=lambda k: ew[k] + ew_cost(k, free, kind))
        ew[e] += ew_cost(e, free, kind)
        return e

    def bal_copy(dst, src, free, allowed=("act", "dve")):
        e = ew_pick(free, allowed)
        if e == "act":
            nc.scalar.copy(dst, src)
        elif e == "dve":
            nc.vector.tensor_copy(dst, src)
        else:
            nc.gpsimd.tensor_copy(dst, src)

    def bal_scale(dst, src, mult, free):
        e = ew_pick(free)
        if e == "act":
            nc.scalar.activation(dst, src, AF.Copy, scale=mult)
        else:
            nc.vector.tensor_scalar(dst, src, mult, None, ALU.mult)

    def bal_exp(atv, scv, free):
        e = ew_pick(free)
        if e == "act":
            nc.scalar.activation(atv, scv, AF.Exp, scale=SCALE / QSC)
        else:
            nc.vector.tensor_scalar(
                atv.bitcast(I16), scv, SCH_A, SCH_B, ALU.mult, ALU.add
            )

    def bal_tt(dst, a, b, free, op=ALU.mult, allowed=("dve", "pool")):
        e = ew_pick(free, allowed, kind="add" if op == ALU.add else "copy")
        if e == "dve":
            nc.vector.tensor_tensor(dst, a, b, op)
        else:
            nc.gpsimd.tensor_tensor(dst, a, b, op)

    with tile.TileContext(nc) as tc, ExitStack() as ctx:
        wp = ctx.enter_context(tc.tile_pool(name="wp", bufs=1))
        small_p = ctx.enter_context(tc.tile_pool(name="small_p", bufs=8))
        at2_p = ctx.enter_context(tc.tile_pool(name="at2_p", bufs=24))
        outp = ctx.enter_context(tc.tile_pool(name="outp", bufs=4))
        # ---- unified PSUM: 4 persistent accumulator banks + ONE 4-slot
        # rotating pool (2KB slots) shared by every transient PSUM tile.
        # No setup/attention phase split -> no drain barrier: local
        # attention overlaps the collective and the qh prep.
        psC = ExitStack()
        ps_acc = psC.enter_context(tc.tile_pool(name="ps_acc", bufs=1, space="PSUM"))
        ps_all = psC.enter_context(tc.tile_pool(name="ps_all", bufs=4, space="PSUM"))
        dramp = ctx.enter_context(tc.tile_pool(name="dramp", bufs=1, space="DRAM"))

        accT = [
            ps_acc.tile([128, 510], F32, name=f"acc{t}", tag=f"acc{t}")
            for t in range(4)
        ]

        def pst(shape):
            return ps_all.tile(shape, F32, tag="sc", name="pst")

        # ---- persistent SBUF tensors ----
        ident = wp.tile([128, 128], F32)
        make_identity(nc, ident[:])

        WkvT = wp.tile([128, 640], BF16)     # [Wk1.T | Wv1.T] interleaved per chunk
        WqT = wp.tile([128, 320], BF16)
        WoT = wp.tile([128, 320], BF16)      # Wo1.T chunks
        Wv2c = wp.tile([64, 640], BF16)      # Wv2 head-major: [d, 32h+r]
        WoT2 = wp.tile([64, 640], BF16)      # Wo1.T head-major: [d, 32h+s]
        Wq2aug = wp.tile([64, 660], F32)     # [Wq2 | bq] head-major, d on partitions
        Wk2aug = wp.tile([64, 660], F32)
        bk2 = wp.tile([64, 640], F32)        # per head: col 0 = bk_h, cols 1:32 zero
        bv_c = wp.tile([128, 12], BF16)
        bo_row = wp.tile([1, N], F32)
        Wo2Ta = wp.tile([33, N], BF16)       # rows 0:32 Wo2.T, row 32 = Wo2@Wo1@bv + bo
        t1sb = wp.tile([32, 2], BF16)
        wsb2 = wp.tile([128, 320], F32)

        # k1 in DoubleRow fp8 layout, chunk-major so the two k-tiles of each
        # key chunk are CONTIGUOUS (ISA: dual-fp8 ldweights forbids gapped
        # k-tile strides): chunk c at cols 256c = [kt0(128 keys) | kt1(128)].
        # kt0 = ranks 0..31 of k1; kt1 = [ones row, zeros...]; chunk 2 key
        # cols 119:128 are zero pads (their scores are finite junk, never
        # consumed by attn@Z which slices [:p]). The 32 rows are REPLICATED
        # into 4 partition bands (32g) so head h can read its lhsT at the
        # same partition base as its qh band (quad-stacked qh evac).
        k1f8 = [
            wp.tile([128, 256 * NIC], FP8, name=f"k1f8{m}", tag=f"k1f8{m}")
            for m in range(QP)
        ]
        k1sb = [
            wp.tile([32, SQP], FP8, name=f"k1sb{bi}", tag=f"k1sb{bi}")
            for bi in range(2)
        ]  # rank-major k1 staging for the cc (two local blocks)
        v1b = [
            wp.tile([32, SQP], BF16, name=f"v1b{m}", tag=f"v1b{m}") for m in range(QP)
        ]
        Mall = wp.tile([32, 640], BF16)      # Mh side by side
        q1Ta = wp.tile([33, SQP], BF16)      # rows 0:32 q1T, row 32 ones; col SQ zero
        o1Ta = wp.tile([33, SQP], BF16)
        Zt = [wp.tile([128, 680], BF16, name=f"Zt{j}", tag=f"Zt{j}") for j in range(NJ)]
        # Zt layout: 20 heads x [32 z-cols | 1.0 | 0.0]; the ones col feeds
        # the softmax denominator through the same accumulating matmul.
        qh_f8 = [
            wp.tile([128, 2 * SQE], FP8, name=f"qh{hp}", tag=f"qh{hp}")
            for hp in range(H // 4)
        ]
        Yg = [wp.tile([128, 480], F32, name=f"Yg{t}", tag=f"Yg{t}") for t in range(4)]
        rrt = wp.tile([128, 64], F32)        # reciprocals, 16 per group
        o1grp = wp.tile([128, 96], F32)      # [i, ic, r] final o1 (pre-transpose)
        scr = wp.tile([128, 384], F32)       # reduce scratch (4 x 96)


        def kw(m, c, g):
            # DoubleRow lhsT for block m, key chunk c, band g: [32, 2, 128]
            return k1f8[m][32 * g : 32 * g + 32, 256 * c : 256 * c + 256].rearrange(
                "p (two s) -> p two s", s=128
            )

        def q3(h):
            hp, g = h // 4, h % 4
            return qh_f8[hp][32 * g : 32 * g + 32, :].rearrange(
                "p (two s) -> p two s", s=SQE
            )

        def k_fill_band0(m, eng, eng2):
            # chunk-major band 0 from the rank-major staging tile (SBUF DMA)
            eng.dma_start(
                k1f8[m][0:32, 0:512].rearrange("p (c s) -> p c s", s=256)[
                    :, :, 0:128
                ],
                k1sb[m][:, 0:256].rearrange("p (c s) -> p c s", s=128),
            )
            eng2.dma_start(k1f8[m][0:32, 512 : 512 + 119], k1sb[m][:, 256:SQ])

        def k_replicate(m, eng):
            # copy band 0 (incl kt1 ones/zeros) into bands 1..3
            for g in range(1, QP):
                eng.dma_start(
                    k1f8[m][32 * g : 32 * g + 32, :], k1f8[m][0:32, :]
                )

        def build_z(j):
            j0, p = SCH[j]
            mp_, off_ = j0 // SQ, j0 % SQ
            for half in range(2):
                zp = pst([128, 512])
                mm(
                    zp[:p, 0:320],
                    v1b[mp_][:, off_ : off_ + p],
                    Mall[:, 320 * half : 320 * half + 320],
                )
                zdst = Zt[j][:p, 340 * half : 340 * half + 340].rearrange(
                    "p (h c) -> p h c", c=34
                )[:, :, 0:32]
                zsrc = zp[:p, 0:320].rearrange("p (h c) -> p h c", c=32)
                bal_copy(zdst, zsrc, 320)

        # ================= setup: x, k/v, collective =================
        with tc.tile_pool(name="wload", bufs=3) as wload, tc.tile_pool(
            name="xT_p", bufs=1
        ) as xT_p:
            # x^T: one tile PER query chunk (separate tiles -> the three XBAR
            # transposes are independent, no WAW serialization). xTa[ic] =
            # [128 f-partitions, 10 c-blocks x 128 queries]; x arrives bf16
            # (host cast, padded to 384 rows) so the XBAR reads DRAM.
            xTa = [
                xT_p.tile([128, N], BF16, name=f"xTa{ic}", tag=f"xTa{ic}")
                for ic in range(2 * NIC)
            ]
            with tc.high_priority():
                # x arrives HOST-TRANSPOSED (feature-major, bf16): row
                # 128*ic+p holds [c, s] -> x[128*ic+s, 128*c+p], for TWO
                # query blocks (own + next neighbor). Plain DMAs, no XBAR.
                # Block A chunks + the k/v weight loads go FIRST so the
                # kv-A -> collective chain is not stuck behind block B's
                # transfers on the serial DMA device.
                for ic in range(2):
                    eng = (nc.sync, nc.scalar)[ic % 2]
                    eng.dma_start(xTa[ic][:, :], xb[128 * ic : 128 * ic + 128, :])
                wsb_k = wload.tile([32, N], F32, tag="wsb_k")
                nc.sync.dma_start(wsb_k[:], Wk1[:])
                wsb_v = wload.tile([32, N], F32, tag="wsb_v")
                nc.scalar.dma_start(wsb_v[:], Wv1[:])
                nc.sync.dma_start(xTa[2][:, :], xb[256:384, :])
            for ic in range(NIC, 2 * NIC):
                eng = (nc.sync, nc.scalar)[ic % 2]
                eng.dma_start(xTa[ic][:, :], xb[128 * ic : 128 * ic + 128, :])

            wsb_q = wload.tile([32, N], F32, tag="wsb_q")
            wsb_o = wload.tile([32, N], F32, tag="wsb_o")

            def xmov(bi, ic, c):
                # moving operand: queries 128*ic.. of feature chunk c
                w = min(128, SQP - 128 * ic)
                return xTa[NIC * bi + ic][:, 128 * c : 128 * c + w]

            # PE p-state warm-up
            warm_ps = pst([128, 320])
            for _ in range(18):
                mm(warm_ps[:, 0:128], ident[:], ident[:], is_transpose=True)

            # --- Wk1/Wv1 -> WkvT (f32 transposes, bf16 evacs) ---
            with tc.high_priority():
                for wi, wsb in enumerate((wsb_k, wsb_v)):
                    tp = pst([128, 320])
                    for c in range(NC10):
                        mm(
                            tp[:, 32 * c : 32 * c + 32],
                            wsb[:, 128 * c : 128 * c + 128],
                            ident[:32, :32],
                            is_transpose=True,
                        )
                    dst = WkvT[:, :].rearrange(
                        "p (c two r) -> p c two r", two=2, r=32
                    )[:, :, wi]
                    bal_copy(dst, tp[:, :].rearrange("p (c r) -> p c r", r=32), 320)

                # --- local k1/v1, block A first ([64, SQP] psum: k rows
                # 0:32, v 32:64). Moving split per (c, ic): each matmul
                # waits only ONE xTa. Only block A feeds the collective --
                # block B (qi+1) is computed locally by every core, so the
                # gather carries just one block per core and can launch
                # after a single kv pass.
                cc_in = dramp.tile([96, SQ], U8)
                cc_out = dramp.tile([96 * QP, SQ], U8)
                for bi in range(2):
                    kv_ps = pst([64, SQP])
                    for c in range(NC10):
                        for ic in range(NIC):
                            w = min(128, SQP - 128 * ic)
                            mm(
                                kv_ps[:, 128 * ic : 128 * ic + w],
                                WkvT[:, 64 * c : 64 * c + 64],
                                xmov(bi, ic, c),
                                start=(c == 0 and ic == 0),
                                stop=(c == NC10 - 1 and ic == NIC - 1),
                                skip_group_check=True,
                            )
                    # staging evacs, single-writer tiles -> clean cc deps
                    nc.vector.tensor_copy(k1sb[bi][:, 0:SQ], kv_ps[0:32, 0:SQ])
                    nc.scalar.copy(v1b[bi][:, 0:SQ], kv_ps[32:64, 0:SQ])
                    if bi == 0:
                        nc.sync.dma_start(
                            cc_in[0:32, :], k1sb[0][:, 0:SQ].bitcast(U8)
                        )
                        nc.scalar.dma_start(
                            cc_in[32:96, :], v1b[0][:, 0:SQ].bitcast(U8)
                        )
                        nc.gpsimd.collective_compute(
                            "AllGather",
                            ALU.bypass,
                            replica_groups=[
                                list(range(g * QP, (g + 1) * QP))
                                for g in range(NCORES // QP)
                            ],
                            ins=[cc_in[:].opt()],
                            outs=[cc_out[:].opt()],
                        )

            nc.sync.dma_start(wsb_q[:], Wq1[:])
            nc.scalar.dma_start(wsb_o[:], Wo1[:])
            # local k1f8 band 0 + kt1 memsets + replicate
            k_fill_band0(0, nc.sync, nc.sync)
            k_fill_band0(1, nc.scalar, nc.scalar)
            for m in range(QP):
                kt1v = k1f8[m][0:32, :].rearrange("p (c s) -> p c s", s=256)[
                    :, :, 128:256
                ]
                nc.gpsimd.memset(kt1v, 0.0)
                nc.gpsimd.memset(kt1v[0:1], 1.0)
                nc.gpsimd.memset(k1f8[m][0:32, 512 + 119 : 512 + 128], 0.0)
            k_replicate(0, nc.sync)
            k_replicate(1, nc.scalar)

            # --- gpsimd work queued behind the collective issue ---
            nc.gpsimd.dma_start(
                Wv2c[:, :].rearrange("d (h r) -> d h r", r=32),
                Wv2[:].rearrange("(h d) r -> d h r", d=64),
            )
            nc.gpsimd.memset(bk2[:], 0.0)
            nc.gpsimd.memset(q1Ta[32:33, :], 1.0)
            if SQP > SQ:
                nc.gpsimd.memset(q1Ta[0:33, SQ:SQP], 0.0)
            for j in range(NJ):
                z3 = Zt[j][:, :].rearrange("p (h c) -> p h c", c=34)
                nc.gpsimd.memset(z3[:, :, 32:33], 1.0)
                nc.gpsimd.memset(z3[:, :, 33:34], 0.0)
            for hp in range(H // 4):
                q4 = qh_f8[hp][:, :].rearrange("p (two s) -> p two s", s=SQE)
                nc.gpsimd.memset(q4[:, :, SQP:SQE], 0.0)
            nc.gpsimd.memset(o1Ta[32:33, :], 1.0)
            if SQP > SQ:
                nc.gpsimd.memset(o1Ta[0:33, SQ:SQP], 0.0)
            nc.gpsimd.memset(bv_c[:], 0.0)
            nc.gpsimd.dma_start(
                bv_c[:, 0:10], bv[:].rearrange("(c p) -> p c", p=128)
            )

            nc.sync.dma_start(
                Wq2aug[:, :].rearrange("d (h r) -> d h r", r=33)[:, :, 0:32],
                Wq2[:].rearrange("(h d) r -> d h r", d=64),
            )
            nc.scalar.dma_start(
                Wk2aug[:, :].rearrange("d (h r) -> d h r", r=33)[:, :, 0:32],
                Wk2[:].rearrange("(h d) r -> d h r", d=64),
            )
            nc.sync.dma_start(
                Wq2aug[:, :].rearrange("d (h r) -> d h r", r=33)[:, :, 32:33],
                bq[:].rearrange("(h d) -> d h", d=64).unsqueeze(2),
            )
            nc.scalar.dma_start(
                Wk2aug[:, :].rearrange("d (h r) -> d h r", r=33)[:, :, 32:33],
                bk[:].rearrange("(h d) -> d h", d=64).unsqueeze(2),
            )
            bal_copy(
                bk2[:, :].rearrange("d (h c) -> d h c", c=32)[:, :, 0:1],
                Wk2aug[:, :].rearrange("d (h r) -> d h r", r=33)[:, :, 32:33],
                20,
            )
            # remote k1/v1: gathered block (qi+mp)%4 at cc rows b*96
            qi = nc.sync.partition_id() % QP
            qi2 = nc.scalar.partition_id() % QP
            for mp in range(2, QP):
                b = (qi + mp) % QP
                b2 = (qi2 + mp) % QP
                nc.sync.dma_start(
                    k1f8[mp][0:32, 0:512].rearrange("p (c s) -> p c s", s=256)[
                        :, :, 0:128
                    ].bitcast(U8),
                    cc_out[bass.DynSlice(b * 96, 32), 0:256],
                )
                nc.sync.dma_start(
                    k1f8[mp][0:32, 512 : 512 + 119].bitcast(U8),
                    cc_out[bass.DynSlice(b * 96, 32), 256:SQ],
                )
                k_replicate(mp, nc.sync)
                nc.scalar.dma_start(
                    v1b[mp][:, 0:SQ].bitcast(U8),
                    cc_out[bass.DynSlice(b2 * 96 + 32, 64), :],
                )

            nc.sync.dma_start(bo_row[:], bo[:].unsqueeze(0))
            nc.sync.dma_start(
                wsb2[:].rearrange("p (c r) -> p c r", r=32),
                Wo2[:].rearrange("(c p) r -> p c r", p=128),
            )

            # --- q-side prep ---
            tp = pst([128, 320])
            for c in range(NC10):
                mm(
                    tp[:, 32 * c : 32 * c + 32],
                    wsb_q[:, 128 * c : 128 * c + 128],
                    ident[:32, :32],
                    is_transpose=True,
                )
            bal_copy(WqT[:], tp[:], 320)
            q1ps = pst([32, SQP])
            for c in range(NC10):
                for ic in range(NIC):
                    w = min(128, SQP - 128 * ic)
                    mm(
                        q1ps[:, 128 * ic : 128 * ic + w],
                        WqT[:, 32 * c : 32 * c + 32],
                        xmov(0, ic, c),
                        start=(c == 0 and ic == 0),
                        stop=(c == NC10 - 1 and ic == NIC - 1),
                        skip_group_check=True,
                    )
            nc.vector.tensor_copy(q1Ta[0:32, :], q1ps[:])

            # --- Mall: Mh = Wv2h @ Wo1_h.T ---
            tp = pst([128, 320])
            for c in range(NC10):
                mm(
                    tp[:, 32 * c : 32 * c + 32],
                    wsb_o[:, 128 * c : 128 * c + 128],
                    ident[:32, :32],
                    is_transpose=True,
                )
            bal_copy(WoT[:], tp[:], 320)
            # head-major copy for the Mall matmuls (all partitions 0:64)
            w2v = WoT2[:, :].rearrange("d (hc two r) -> d hc two r", two=2, r=32)
            t3v = tp[:, :].rearrange("p (c r) -> p c r", r=32)
            nc.vector.tensor_copy(w2v[:, :, 0], t3v[0:64, :, :])
            nc.scalar.copy(w2v[:, :, 1], t3v[64:128, :, :])
            for half in range(2):
                mall_ps = pst([32, 320])
                for hh in range(10):
                    h = 10 * half + hh
                    mm(
                        mall_ps[:, 32 * hh : 32 * hh + 32],
                        Wv2c[:, 32 * h : 32 * h + 32],
                        WoT2[:, 32 * h : 32 * h + 32],
                        skip_group_check=True,
                    )
                bal_copy(Mall[:, 320 * half : 320 * half + 320], mall_ps[:], 320)

            # local Z chunks: both local blocks (need Mall + local v1 only)
            for j in range(2 * NIC):
                build_z(j)

            def attend(J, h):
                j0, p = SCH[J]
                mp = j0 // SQ
                # full 128-key lhsT even for the 119-wide chunk: pad keys
                # are zero -> finite junk scores in rows p:128, unused
                sc = pst([128, 512])
                mm(
                    sc[0:128, 0:SQE],
                    kw(mp, J % NIC, h % 4),
                    q3(h)[:, :, :],
                    perf_mode=DR,
                    tile_position=(32 * (h % 4), 0),
                )
                at2 = at2_p.tile([128, SQE], BF16, tag="at")
                bal_exp(at2[:p, :], sc[:p, 0:SQE], SQE)
                t, base = h // 5, (h % 5) * 3
                for ic, (i0, iw) in enumerate(ICH):
                    c0 = 34 * (base + ic)
                    # az reads a full 128-query stripe (query pads are
                    # finite junk) so every acc partition gets written
                    mm(
                        accT[t][0:128, c0 : c0 + 34],
                        at2[:p, i0 : i0 + 128],
                        Zt[J][:p, 34 * h : 34 * h + 34],
                        start=(J == 0 and h % 5 == 0 and ic == 0),
                        stop=(J == NJ - 1 and h % 5 == 4 and ic == 2),
                        skip_group_check=True,
                    )
                if J == NJ - 1 and h % 5 == 4:
                    # tile h//5 complete: divide by the softmax denominator
                    # now, while the remaining exps still run
                    td = h // 5
                    acc3 = accT[td][:, :].rearrange("p (s c) -> p s c", c=34)
                    nc.vector.reciprocal(
                        rrt[:, 16 * td : 16 * td + 15].unsqueeze(2),
                        acc3[:, :, 32:33],
                    )
                    ew["dve"] += 141.0
                    y_ap = Yg[td][:, :].rearrange("p (s c) -> p s c", c=32)
                    rb, ab = bass.broadcast_tensor_aps(
                        rrt[:, 16 * td : 16 * td + 15].unsqueeze(2),
                        acc3[:, :, 0:32],
                    )
                    bal_tt(y_ap, ab, rb, 480, ALU.mult, allowed=("dve",))
                    # head-sum partial for this group tile, immediately
                    rsrc = Yg[td][:, :].rearrange(
                        "p (hl ic c) -> p ic c hl", hl=5, ic=3, c=32
                    )
                    rdst = scr[:, 96 * td : 96 * td + 96].rearrange(
                        "p (ic c) -> p ic c", c=32
                    )
                    nc.vector.tensor_reduce(
                        rdst, rsrc, mybir.AxisListType.X, ALU.add
                    )
                    ew["dve"] += 625.0

            # --- wm + qh (four heads stacked in partition bands 32g), each
            # quad IMMEDIATELY followed by its local-block attention so the
            # exp engines fill the collective window ---
            for hp in range(H // 4):
                # phase 1: the four wm coefficient tiles (short-lived psum)
                wms = []
                for g in range(4):
                    h = 4 * hp + g
                    wmps = pst([33, 64])
                    mm(
                        wmps[0:32, 0:32],
                        Wq2aug[:, 33 * h : 33 * h + 32],
                        Wk2aug[:, 33 * h : 33 * h + 32],
                    )
                    mm(
                        wmps[32:33, 0:32],
                        Wq2aug[:, 33 * h + 32 : 33 * h + 33],
                        Wk2aug[:, 33 * h : 33 * h + 32],
                        skip_group_check=True,
                    )
                    # cols 32:64 = kt1 coefficients: col 32 = bk (ones-row
                    # coefficient), cols 33:64 = 0 (zero cols of bk2)
                    mm(
                        wmps[0:33, 32:64],
                        Wq2aug[:, 33 * h : 33 * h + 33],
                        bk2[:, 32 * h : 32 * h + 32],
                        skip_group_check=True,
                    )
                    wm = small_p.tile([33, 64], BF16, tag="wm_sb")
                    bal_scale(wm[:], wmps[:], QSC, 64)
                    wms.append(wm)
                # phase 2: band-stacked qh matmuls + two 128-partition evacs
                qhA = pst([128, 512])
                qhB = pst([128, 512])
                for g in range(4):
                    mm(
                        qhA[32 * g : 32 * g + 32, 0:SQP],
                        wms[g][:, 0:32],
                        q1Ta[:],
                        skip_group_check=(g > 0),
                        tile_position=(0, 32 * g),
                    )
                    mm(
                        qhB[32 * g : 32 * g + 32, 0:SQP],
                        wms[g][:, 32:64],
                        q1Ta[:],
                        skip_group_check=(g > 0),
                        tile_position=(0, 32 * g),
                    )
                qdst = qh_f8[hp][:, :].rearrange("p (two s) -> p two s", s=SQE)
                bal_copy(qdst[:, 0, 0:SQP], qhA[:, 0:SQP], SQP)
                bal_copy(qdst[:, 1, 0:SQP], qhB[:, 0:SQP], SQP)

            # local attention en bloc (quads all ready; scores get the full
            # 4-slot rotation depth)
            for hp in range(H // 4):
                for J in range(2 * NIC):
                    for g in range(4):
                        attend(J, 4 * hp + g)

            # --- Wo2 -> Wo2Ta (bf16) + bo_eff into row 32 (tail prep) ---
            for g0 in range(0, NC10, 4):
                gn = min(4, NC10 - g0)
                tp2 = pst([32, 512])
                for k in range(gn):
                    c = g0 + k
                    mm(
                        tp2[:, 128 * k : 128 * k + 128],
                        wsb2[:, 32 * c : 32 * c + 32],
                        ident[:],
                        is_transpose=True,
                    )
                bal_copy(
                    Wo2Ta[0:32, 128 * g0 : 128 * (g0 + gn)],
                    tp2[:, : 128 * gn],
                    512,
                )
            t1ps = pst([32, 2])
            for c in range(NC10):
                mm(
                    t1ps[:],
                    WoT[:, 32 * c : 32 * c + 32],
                    bv_c[:, c : c + 2],
                    start=(c == 0),
                    stop=(c == NC10 - 1),
                )
            nc.vector.tensor_copy(t1sb[:], t1ps[:])
            for (n0, nw) in OSUB:
                beps = pst([1, 512])
                mm(beps[:, :nw], t1sb[:, 0:1], Wo2Ta[0:32, n0 : n0 + nw])
                nc.vector.tensor_add(
                    Wo2Ta[32:33, n0 : n0 + nw],
                    beps[:, :nw],
                    bo_row[:, n0 : n0 + nw],
                )

        # ================= remote attention =================
        for J in range(2 * NIC, NJ):
            if J == 2 * NIC:
                build_z(2 * NIC)
                build_z(2 * NIC + 1)
            if J + 2 < NJ:
                build_z(J + 2)
            for h in range(H):
                attend(J, h)

        # ================= tail: epilogue + out-projection ======
        psC.close()
        psD = ExitStack()
        psf = psD.enter_context(tc.tile_pool(name="psf", bufs=2, space="PSUM"))

        # add the 4 group partials -> o1grp [i, (ic, r)]
        bal_tt(scr[:, 0:96], scr[:, 0:96], scr[:, 96:192], 96, ALU.add)
        bal_tt(scr[:, 192:288], scr[:, 192:288], scr[:, 288:384], 96, ALU.add)
        bal_tt(o1grp[:], scr[:, 0:96], scr[:, 192:288], 96, ALU.add)

        # out-projection: per-OSUB pipeline (small evacs + small DMAs so
        # transfers overlap compute), bf16 staging halves the DMA bytes
        for ic, (i0, iw) in enumerate(ICH):
            o1ps = psf.tile([32, SQP], F32, tag="o1t", bufs=3)
            mm(
                o1ps[:, i0 : i0 + iw],
                o1grp[0:iw, 32 * ic : 32 * ic + 32],
                ident[:iw, :iw],
                is_transpose=True,
            )
            bal_copy(o1Ta[0:32, i0 : i0 + iw], o1ps[:, i0 : i0 + iw], iw)
            osb = outp.tile([128, N], BF16, tag="osb")
            for oi, (n0, nw) in enumerate(OSUB):
                fps = psf.tile([128, 512], F32, tag="fp", bufs=5)
                mm(fps[:iw, :nw], o1Ta[:, i0 : i0 + iw], Wo2Ta[:, n0 : n0 + nw])
                bal_copy(osb[:iw, n0 : n0 + nw], fps[:iw, :nw], nw)
                eng = (nc.sync, nc.gpsimd, nc.scalar)[(3 * ic + oi) % 3]
                eng.dma_start(out[i0 : i0 + iw, n0 : n0 + nw], osb[:iw, n0 : n0 + nw])

        psD.close()

    nc.compile()
    return nc


_NC_CACHE = {}


def _get_nc(S, SQ):
    key = (S, SQ)
    if key not in _NC_CACHE:
        _NC_CACHE[key] = build_nc(S, SQ)
    return _NC_CACHE[key]


def kernel(**inputs):
    from concourse.bass_utils import run_bass_kernel_spmd

    x = np.asarray(inputs["x"], dtype=np.float32)
    B, S, n = x.shape
    assert n == N and B * QP == NCORES
    SQ = S // QP
    nc = _get_nc(S, SQ)

    wnames = [
        "Wq1", "Wq2", "bq", "Wk1", "Wk2", "bk",
        "Wv1", "Wv2", "bv", "Wo1", "Wo2", "bo",
    ]
    weights = {
        k: np.ascontiguousarray(np.asarray(inputs[k], dtype=np.float32))
        for k in wnames
    }

    import ml_dtypes

    SQPAD = 128 * ((SQ + 127) // 128)
    NICW = SQPAD // 128
    in_maps = []
    for core in range(NCORES):
        b, qi = divmod(core, QP)
        xpad = np.zeros((2 * SQPAD, N), dtype=np.float32)
        xpad[:SQ] = x[b, SQ * qi : SQ * (qi + 1)]
        q2 = (qi + 1) % QP
        xpad[SQPAD : SQPAD + SQ] = x[b, SQ * q2 : SQ * (q2 + 1)]
        # feature-major host transpose: xbt[128*ic + p, 128*c + s]
        # = x[128*ic + s, 128*c + p]  (see xTa layout in build_nc)
        xbt = (
            xpad.reshape(2 * NICW, 128, N // 128, 128)
            .transpose(0, 3, 2, 1)
            .reshape(2 * SQPAD, N)
            .astype(ml_dtypes.bfloat16)
        )
        m = {"xb": np.ascontiguousarray(xbt)}
        m.update(weights)
        in_maps.append(m)

    res = run_bass_kernel_spmd(nc, in_maps, core_ids=list(range(NCORES)))
    outs = res.results if hasattr(res, "results") else res

    out = np.zeros((B, S, N), dtype=np.float32)
    for core in range(NCORES):
        b, qi = divmod(core, QP)
        out[b, SQ * qi : SQ * (qi + 1), :] = outs[core]["out"].astype(np.float32)
    return out
